# revision 25
# baseline (speedup 1.0000x reference)
"""Trainium2 Bass kernel for EnhancedSpikingAudioNet (4-layer LIF SNN).

Network (eval mode): for t in 0..99:
    s1,m1 = LIF(x_t @ W1.T + b1, m1)
    s2,m2 = LIF(s1 @ W2.T + b2, m2)
    s3,m3 = LIF(s2 @ W3.T + b3, m3)
    s4,m4 = LIF(s3 @ W4.T + b4, m4)
returns m4 (final step), shape [B=256, 10].

LIF (snnTorch Leaky, reset_mechanism='subtract', beta=.95, thr=1):
    reset = (m_prev > 1);  m = beta*m_prev + cur - reset;  s = (m > 1)
Note reset(t) == spike(t-1), so only s needs materializing.

Strategy: data-parallel over batch (32 per core, 8 cores).  Inside a
core, time is blocked with ALTERNATING block sizes [12,8]x5 (sum 100):
all matmuls for a block are batched over its steps (moving free dim
N = TB*32 = 384 or 256, both >= the 256-row f32r full-rate cliff); only
the per-step LIF update is sequential (3 DVE/Pool ops per layer per
step).  The LAST block is 8 steps, shortening the exposed tail LIF
chains.  Layout: features on partitions (128-chunks), (t, batch) on the
free dim.  PSUM is drained to SBUF by ScalarE with the layer bias
fused in.

Head: the DMA transfers serialize in queue order, so the queue is
ordered by first use: x-block-0 and the W1 planes are split into
k-quarters and interleaved (xh-q, w1h-q, xl-q, w1l-q per quarter),
then x1, W2..W4.  Layer-1 matmuls keep the baseline's k-major
term order per PSUM group (k ascending; wh@xh, wh@xl, wl@xh per k --
accumulation order defines the fp32 rounding, and the spike cascade
amplifies any change, so this order is load-bearing for accuracy) but
hold all six output-chunk PSUM banks open at once, so the PE starts as
soon as the first k-quarter lands instead of waiting for all of x0+W1
(36us of serial DMA -> first matmul at ~7.6us).

Tail: the pipeline-drain region (after the last x block) is bounded by
the serial chain lif1@last -> mm2 -> lif2@last -> mm3 -> lif3@last ->
mm4 -> lif4@last.  The last blocks are 8 steps (shorter chains), the
small layer-3/4 tail chains run entirely on DVE (no Pool round-trip on
the critical path), and layer 4's dead final spike is skipped.

Numerics: the spike cascade amplifies matmul noise (a plain f32r
matmul gives ~16% output error; even exact-fp32 summation-order noise
gives ~1.6%), so matmuls must be fp32-faithful.  Hardware probing
established: float32r = operands rounded RNE to 11 mantissa bits
(FP22 e10m11), then EXACT products with clean fp32 accumulation (a
pre-rounded-operand probe matches an exact model to 1e-7), at full PE
rate (1 cyc/row) for moving dims >= 256.  Therefore every fp32 tensor
is split host-side into two 11-bit planes (h = rne11(a), l = a-h; the
residual fits 12 significand bits, so a == h+l exactly and both planes
survive the hw operand rounding unchanged).  Weights use 2 planes;
spikes are 0/1 (f32r-exact, single copy); x uses 2 planes.  Layer 1
accumulates wh@xh + wh@xl + wl@xh (dropping wl@xl ~ 2^-24), layers 2-4
accumulate wh@s + wl@s.  Everything streams at 1 cycle/row and the
result is fp32-exact (hw-validated: bit-identical to the jax CPU
reference).
"""

import os
import sys

import numpy as np

for _p in ("/opt/trn_rl_repo",):
    if os.path.isdir(_p) and _p not in sys.path:
        sys.path.insert(0, _p)

import concourse.bass as bass
import concourse.mybir as mybir
import concourse.tile as tile
from concourse import bass_utils

F32 = mybir.dt.float32
F32R = mybir.dt.float32r
ALU = mybir.AluOpType
ACTF = mybir.ActivationFunctionType
PLANES = ("h", "l")  # 11-bit f32r planes


def _patch_tail_drain():
    """This container's walrus allows only ONE sync-wait on a Drain
    instruction; Tile's kernel-tail drain can carry several (one per DMA
    HW queue).  Spread the waits across consecutive drains instead."""
    from concourse.vector_clock import ScopedClock

    if getattr(tile.TileContext, "_tail_drain_patched", False):
        return

    def _drain_and_barrier(self, tick_clock, wait_clock):
        drain_inst = self.nc.sync.drain()
        wait_clock.add_sem_waits(
            drain_inst.ins, ScopedClock({None: tick_clock.global_clock})
        )
        si = drain_inst.ins.sync_info
        if si is not None and si.on_wait and len(si.on_wait) > 1:
            waits = list(si.on_wait)
            drain_inst.ins.sync_info = mybir.SyncInfo(
                on_wait=[waits[0]], on_update=list(si.on_update or [])
            )
            for w in waits[1:]:
                extra = self.nc.sync.drain()
                extra.ins.sync_info = mybir.SyncInfo(on_wait=[w], on_update=[])

        self.nc.all_engine_barrier()
        assert self.sems is not None
        popped = self.nc._tile_sem_poison_stack.pop()
        assert popped is self._sem_poison
        self.nc.clear_and_free_semaphores(
            list(self.sems.allocated().values())
        )
        self.nc.all_engine_barrier()

    tile.TileContext._drain_and_barrier = _drain_and_barrier
    tile.TileContext._tail_drain_patched = True


_patch_tail_drain()


def _split_multi_waits(nc):
    """This walrus build rejects instructions carrying more than one
    sync-wait (a DMA-HW-queue sem wait expands into several wait
    commands).  Give every instruction at most one wait; extras go onto
    same-engine NOPs inserted immediately before it."""

    def fresh_nop(engine):
        eng = nc.engines[engine]
        bi = eng.nop(nofuse=True)
        raw = bi.ins
        # nop() appended raw to the current bb -- remove it, we re-insert.
        for bb in nc.main_func.blocks:
            try:
                bb.instructions.remove(raw)
                break
            except ValueError:
                continue
        return raw

    for bb in nc.main_func.blocks:
        insts = bb.instructions
        i = 0
        while i < len(insts):
            ins = insts[i]
            si = getattr(ins, "sync_info", None)
            ow = list(si.on_wait) if (si is not None and si.on_wait) else []
            if len(ow) > 1:
                upd = list(si.on_update or [])
                for w in ow[:-1]:
                    nop = fresh_nop(ins.engine)
                    nop.sync_info = mybir.SyncInfo(on_wait=[w], on_update=[])
                    insts.insert(i, nop)
                    i += 1
                ins.sync_info = mybir.SyncInfo(on_wait=[ow[-1]],
                                               on_update=upd)
            i += 1


T, B, D = 100, 256, 1024
HH = [1024, 768, 512, 256, 10]  # H[l-1] -> H[l] for layer l in 1..4
NCORES = 8
BC = B // NCORES  # 32 batch per core
BLOCKS = [12, 8, 12, 8, 12, 8, 12, 8, 12, 8]  # per-block steps, sum 100
NBLK = len(BLOCKS)
PREFIX = [0]
for _tb in BLOCKS:
    PREFIX.append(PREFIX[-1] + _tb)
assert PREFIX[-1] == T
RING = 20                 # ring slots; even blocks at base 0, odd at 12
TBMAX = max(BLOCKS)       # 12
NBMAX = TBMAX * BC        # 384
BETA = 0.95


def _base(b):  # ring slot base for block b
    return 0 if b % 2 == 0 else 12


def _kch(l):  # contraction chunks for layer l (input feature chunks)
    return (HH[l - 1] + 127) // 128


def _mch(l):  # output feature chunks
    return (HH[l] + 127) // 128


def _mpart(l):  # partitions used by last output chunk
    r = HH[l] % 128
    return 128 if r == 0 else r


def build_nc(repeat=1):
    nc = bass.Bass(target_bir_lowering=False, trn_type="TRN2")

    x_d = {
        p: nc.dram_tensor(f"x_{p}", [D, T * BC], F32R,
                          kind="ExternalInput") for p in PLANES
    }
    w_d = {}
    b_d = {}
    for l in range(1, 5):
        for p in PLANES:
            w_d[l, p] = nc.dram_tensor(
                f"w{l}{p}", [_kch(l), 128, HH[l]], F32R,
                kind="ExternalInput"
            )
        b_d[l] = nc.dram_tensor(f"b{l}", [HH[l]], F32, kind="ExternalInput")
    out_d = nc.dram_tensor("out", [10, BC], F32, kind="ExternalOutput")

    with tile.TileContext(nc) as tc:
        from contextlib import ExitStack

        with ExitStack() as ctx:
            wpool = ctx.enter_context(tc.tile_pool(name="weights", bufs=1))
            xpool = ctx.enter_context(tc.tile_pool(name="xblk", bufs=2))
            spool = ctx.enter_context(tc.tile_pool(name="state", bufs=1))
            # PSUM: 8 banks, one rotating ring shared by all layers
            # (reuse distance 8 groups keeps drains off the critical
            # path).  Layer-1's six concurrently-open groups take six
            # consecutive ring slots.
            psum = ctx.enter_context(
                tc.tile_pool(name="psum", bufs=8, space="PSUM")
            )

            # ---- SBUF tiles for weights + biases (DMAs emitted below in
            # first-use order; transfers serialize in queue order) ----
            w_sb = {}
            b_sb = {}
            for l in range(1, 5):
                M = HH[l]
                kc = _kch(l)
                for p in PLANES:
                    w_sb[l, p] = wpool.tile([128, kc * M], F32R,
                                            name=f"wsb{l}{p}")
                mp = _mpart(l)
                b_sb[l] = wpool.tile([128, _mch(l)], F32, name=f"bsb{l}")

            def dma_bias(l):
                mp = _mpart(l)
                nc.sync.dma_start(
                    b_sb[l][:mp, :],
                    b_d[l].rearrange("(c q) -> q c", q=mp)
                    if _mch(l) > 1
                    else b_d[l][:].unsqueeze(-1),
                )

            def dma_w(l, p, k0, k1):
                """Weight plane DMA for contraction chunks [k0, k1)."""
                M = HH[l]
                nc.sync.dma_start(
                    w_sb[l, p][:, k0 * M:k1 * M].rearrange(
                        "q (k m) -> q k m", m=M),
                    w_d[l, p][k0:k1].rearrange("k q m -> q k m"),
                )

            # ---- persistent state ----
            m_t = {}    # membranes [128, Fl]  (l4: rows 0..9)
            tmp_t = {}
            s_t = {}    # spike rings, k-major: [128, kchunks * RING * BC]
            c_t = {}    # cur rings, slot-major: [128, RING * Fl]
            for l in range(1, 5):
                mc = _mch(l)
                mp = _mpart(l)
                Fl = mc * BC
                pp = mp if mc == 1 else 128
                m_t[l] = spool.tile([pp, Fl], F32, name=f"mem{l}")
                tmp_t[l] = spool.tile([pp, Fl], F32, name=f"tmp{l}")
                s_t[l] = spool.tile([pp, mc * RING * BC], F32R,
                                    name=f"spk{l}")
                c_t[l] = spool.tile([pp, RING * Fl], F32, name=f"cur{l}")
                nc.vector.memset(m_t[l], 0.0)
                nc.vector.memset(s_t[l].bitcast(F32), 0.0)

            def lif_chain(l, b, engines="dve+pool", skip_last_spike=False):
                """Per-step emit closures for layer l's LIF chain over
                block b.  engines: "dve+pool" = stt/tt on DVE, spike
                threshold on Pool (default); "dve" = all three on DVE
                (lowest latency for an exposed chain); "pool" = all
                three on Pool (offloads the DVE queue entirely -- used
                for the small layers 3/4 whose chains otherwise share
                the DVE with the tail-critical layer-1/2 chains).
                skip_last_spike drops the final step's spike (dead for
                layer 4's last block)."""
                blk = b % NBLK
                TB = BLOCKS[blk]
                mc = _mch(l)
                Fl = mc * BC
                sb = _base(blk)
                pv0 = _base(blk - 1 if blk > 0 else NBLK - 1) + \
                    BLOCKS[blk - 1 if blk > 0 else NBLK - 1] - 1
                mem = m_t[l]
                tmp = tmp_t[l]
                mem3 = mem.rearrange("q (k b) -> q k b", b=BC)
                tmp3 = tmp.rearrange("q (k b) -> q k b", b=BC)
                s4 = s_t[l].rearrange("q (k r b) -> q k r b", r=RING, b=BC)

                main = nc.gpsimd if engines == "pool" else nc.vector
                gt = nc.vector if engines == "dve" else nc.gpsimd

                def step(t):
                    slot = sb + t
                    prev = pv0 if t == 0 else sb + t - 1
                    cur = c_t[l][:, slot * Fl:(slot + 1) * Fl]
                    # tmp = beta*mem + cur
                    main.scalar_tensor_tensor(
                        tmp, mem, BETA, cur, op0=ALU.mult, op1=ALU.add
                    )
                    # mem = tmp - s_prev
                    main.tensor_tensor(
                        mem3, tmp3, s4[:, :, prev, :], op=ALU.subtract
                    )
                    if skip_last_spike and t == TB - 1:
                        return
                    # s[slot] = mem > 1
                    gt.tensor_scalar(
                        s4[:, :, slot, :], mem3, 1.0, None, op0=ALU.is_gt
                    )

                return [lambda t=t: step(t) for t in range(TB)]

            def emit_chains(chains, zipped=False):
                """Emit chains sequentially or round-robin interleaved
                (zipped chains pace each other instead of head-of-line
                blocking the in-order engine queues)."""
                if not zipped:
                    for ch in chains:
                        for st in ch:
                            st()
                    return
                mx = max((len(c) for c in chains), default=0)
                for i in range(mx):
                    for ch in chains:
                        if i < len(ch):
                            ch[i]()

            def lif_steps(l, b):
                emit_chains([lif_chain(l, b)])

            def drain_one(l, blk, m, pp, c4, ps, TB, sb):
                """PSUM -> cur-ring drain with fused bias.  For the
                last block the exposed tail LIF chain gates on ALL m
                drains, so odd chunks drain on DVE in parallel with
                ScalarE (same fp32 add -> bit-identical)."""
                ps3 = ps.rearrange("q (t b) -> q t b", b=BC)
                nc.scalar.activation(
                    c4[:pp, sb:sb + TB, m, :],
                    ps3[:pp, :, :],
                    ACTF.Identity,
                    bias=b_sb[l][:pp, m:m + 1],
                )

            def layer1_matmul(b, xb):
                """Layer-1 matmuls for block b.  All 6 output-chunk
                PSUM groups are open concurrently and terms stream in
                k-major order (k ascending; wh@xh, wh@xl, wl@xh per k
                -- the SAME per-group accumulation order as the
                reference-matching baseline, so numerics are unchanged)
                which lets the PE start as soon as the first k-chunks
                of x/W1 arrive at the head."""
                blk = b % NBLK
                TB = BLOCKS[blk]
                NB = TB * BC
                mc = _mch(1)   # 6
                kc = _kch(1)   # 8
                M = HH[1]
                sb = _base(blk)
                c4 = c_t[1].rearrange("q (r k b) -> q r k b", r=RING, b=BC)
                ps = [psum.tile([128, NB], F32, name=f"ps1_{m}",
                                tag="ps") for m in range(mc)]
                terms = [("h", "h"), ("h", "l"), ("l", "h")]
                for k in range(kc):
                    for ti, (wp, xp) in enumerate(terms):
                        for m in range(mc):
                            lhsT = w_sb[1, wp][:, k * M + m * 128:
                                               k * M + (m + 1) * 128]
                            nc.tensor.matmul(
                                ps[m], lhsT,
                                xb[xp][:, k * NBMAX:k * NBMAX + NB],
                                start=(k == 0 and ti == 0),
                                stop=(k == kc - 1 and ti == 2),
                            )
                for m in range(mc):
                    drain_one(1, blk, m, 128, c4, ps[m], TB, sb)

            def layer_matmul(l, b):
                """Batched spike matmuls for layer l>=2 over block b;
                drains psum to c_t[l] with bias fused."""
                blk = b % NBLK
                TB = BLOCKS[blk]
                NB = TB * BC
                mc = _mch(l)
                kc = _kch(l)
                M = HH[l]
                mp = _mpart(l)
                sb = _base(blk)
                c4 = c_t[l].rearrange("q (r k b) -> q r k b", r=RING, b=BC)
                sl = s_t[l - 1]
                terms = [(k, wp) for k in range(kc) for wp in PLANES]
                for m in range(mc):
                    pp = mp if m == mc - 1 else 128
                    ps = psum.tile([pp, NB], F32, name=f"ps{l}",
                                   tag="ps")
                    for i, (k, wp) in enumerate(terms):
                        lhsT = w_sb[l, wp][:, k * M + m * 128:
                                           k * M + m * 128 + pp]
                        rhs = sl[:, k * RING * BC + sb * BC:
                                 k * RING * BC + sb * BC + NB]
                        nc.tensor.matmul(
                            ps, lhsT, rhs,
                            start=(i == 0), stop=(i == len(terms) - 1),
                        )
                    drain_one(l, blk, m, pp, c4, ps, TB, sb)

            def dma_x(b, split=False):
                """x tile DMA for block b.  split=True emits each plane
                in two k-halves (head fine-grained interleave); the
                caller controls queue order via `parts`."""
                blk = b % NBLK
                TB = BLOCKS[blk]
                NB = TB * BC
                kc = _kch(1)
                col = PREFIX[blk] * BC
                tiles = {}
                parts = {}
                for p in PLANES:
                    xt = xpool.tile([128, kc * NBMAX], F32R,
                                    name=f"xb{p}", tag=f"xb{p}")
                    tiles[p] = xt
                    x3 = xt.rearrange("q (k n) -> q k n", n=NBMAX)

                    def emit(k0, k1, p=p, x3=x3):
                        nc.sync.dma_start(
                            x3[:, k0:k1, :NB],
                            x_d[p][k0 * 128:k1 * 128,
                                   col:col + NB].rearrange(
                                "(k q) n -> q k n", q=128),
                        )
                    parts[p] = emit
                if split:
                    kh = kc // 2
                    return tiles, parts, kh
                for p in PLANES:
                    parts[p](0, kc)
                return tiles

            # ---- head: DMA queue in first-use order.  Layer-1's
            # matmuls consume (xh, xl, w1h, w1l) k-chunk by k-chunk, so
            # the queue interleaves all four in k-quarters: the PE
            # starts after the first quarter instead of after all of
            # x0+W1. ----
            kc1 = _kch(1)
            x_tiles = {}
            x_tiles[0], xparts, _ = dma_x(0, split=True)
            for q in range(4):
                k0, k1 = 2 * q, 2 * q + 2
                xparts["h"](k0, k1)
                dma_w(1, "h", k0, k1)
                xparts["l"](k0, k1)
                dma_w(1, "l", k0, k1)
                if q == 1:
                    for l in range(1, 5):
                        dma_bias(l)
            x_tiles[1] = dma_x(1)
            for l in range(2, 5):
                for p in PLANES:
                    dma_w(l, p, 0, _kch(l))

            # Software pipeline: at tick t, layer l works on block
            # t-(l-1); the PE's matmuls for tick t depend only on LIF
            # work emitted at tick t-1, so the PE never waits on the DVE
            # in steady state.  Repeats (timing runs) extend the range.
            nblk_r = NBLK * repeat
            nticks = nblk_r + 4
            last = nblk_r - 1

            def chain(l, b):
                """lif_chain with engine placement: layers 3/4 run
                entirely on Pool (off the DVE queue, which the
                tail-critical layer-1/2 chains need), except their
                exposed last blocks which run all-DVE for latency.
                The dead final spike of layer 4's last block is
                skipped."""
                if l in (3, 4) and b >= nblk_r - 3:
                    eng = "dve"
                else:
                    eng = "dve+pool"
                return lif_chain(
                    l, b, engines=eng,
                    skip_last_spike=(l == 4 and b == last),
                )

            for tick in range(nticks):
                for l in (1, 2, 3, 4):
                    b = tick - (l - 1)
                    if not (0 <= b < nblk_r):
                        continue
                    if l == 1:
                        layer1_matmul(b, x_tiles[b])
                        x_tiles.pop(b - 2, None)
                        # prefetch x for block b+2 (reuses b's buffer;
                        # emitted AFTER b's reads so the WAR dep exists)
                        if b + 2 < nblk_r:
                            x_tiles[b + 2] = dma_x(b + 2)
                    else:
                        layer_matmul(l, b)
                    if b - 1 >= 0:
                        emit_chains([chain(l, b - 1)])
                    if b == last:
                        # tail: drain this layer's LIF chain now so the
                        # next layer's matmuls (next tick) see spikes
                        # without waiting behind other layers' chains.
                        emit_chains([chain(l, b)])

            nc.sync.dma_start(out_d[:, :], m_t[4])

    _split_multi_waits(nc)
    return nc


_NC_CACHE = None


def _get_nc():
    global _NC_CACHE
    if _NC_CACHE is None:
        _NC_CACHE = build_nc()
    return _NC_CACHE


def _rne11(a):
    """Round fp32 mantissa to 11 bits (RNE) -- the f32r operand grid."""
    u = np.ascontiguousarray(a, np.float32).view(np.uint32).astype(np.uint64)
    zb = 12  # 23 - 11
    lsb = (u >> zb) & 1
    add = lsb + ((1 << (zb - 1)) - 1)
    r = ((u + add) >> zb) << zb
    return r.astype(np.uint32).view(np.float32)


def _split2_11(a):
    """fp32 -> two 11-bit-mantissa planes with h + l == a exactly."""
    a = np.asarray(a, np.float32)
    h = _rne11(a)
    l = (a - h).astype(np.float32)
    return h, l


def prep_inputs(x, W1, b1, W2, b2, W3, b3, W4, b4):
    """Full inputs -> per-core in_maps."""
    Ws = {1: W1, 2: W2, 3: W3, 4: W4}
    bs = {1: b1, 2: b2, 3: b3, 4: b4}
    shared = {}
    for l in range(1, 5):
        wt = np.ascontiguousarray(
            np.asarray(Ws[l], np.float32).T.reshape(_kch(l), 128, HH[l])
        )
        wh, wl = _split2_11(wt)
        shared[f"w{l}h"] = wh
        shared[f"w{l}l"] = wl
        shared[f"b{l}"] = np.ascontiguousarray(bs[l], dtype=np.float32)
    in_maps = []
    for c in range(NCORES):
        xc = np.asarray(x[:, c * BC:(c + 1) * BC, :], np.float32)
        xc = np.ascontiguousarray(xc.transpose(2, 0, 1).reshape(D, T * BC))
        xh, xl = _split2_11(xc)
        m = {"x_h": xh, "x_l": xl}
        m.update(shared)
        in_maps.append(m)
    return in_maps


def run(in_maps, trace=False):
    nc = _get_nc()
    return bass_utils.run_bass_kernel_spmd(
        nc, in_maps, core_ids=list(range(NCORES)), trace=trace
    )


def kernel(**inputs):
    in_maps = prep_inputs(**inputs)
    res = run(in_maps)
    out = np.empty((B, 10), dtype=np.float32)
    for c in range(NCORES):
        out[c * BC:(c + 1) * BC, :] = res.results[c]["out"].T
    return out


def bench(in_maps, iters=20, nc=None):
    """Repeat-execute the kernel via a cached sharded jit; returns list of
    per-call wall times (seconds).  Mirrors bass2jax.run_bass_via_pjrt's
    multi-core path but keeps inputs device-resident across calls."""
    import time

    import jax
    import concourse.mybir as mybir_
    from jax.sharding import Mesh, PartitionSpec
    from jax.experimental.shard_map import shard_map
    from concourse import bass2jax

    bass2jax.install_neuronx_cc_hook()
    if nc is None:
        nc = _get_nc()

    part_name = (nc.partition_id_tensor.name
                 if nc.partition_id_tensor else None)
    in_names, out_names, out_avals, zero_outs = [], [], [], []
    for alloc in nc.m.functions[0].allocations:
        if not isinstance(alloc, mybir_.MemoryLocationSet):
            continue
        name = alloc.memorylocations[0].name
        if alloc.kind == "ExternalInput":
            if name != part_name:
                in_names.append(name)
        elif alloc.kind == "ExternalOutput":
            out_names.append(name)
            shape = tuple(alloc.tensor_shape)
            dtype = mybir_.dt.np(alloc.dtype)
            out_avals.append(jax.core.ShapedArray(shape, dtype))
            zero_outs.append(np.zeros(shape, dtype))
    n_params = len(in_names)
    all_in_names = in_names + out_names
    if part_name is not None:
        all_in_names = all_in_names + [part_name]

    def _body(*args):
        operands = list(args)
        if part_name is not None:
            operands.append(bass2jax.partition_id_tensor())
        outs = bass2jax._bass_exec_p.bind(
            *operands,
            out_avals=tuple(out_avals),
            in_names=tuple(all_in_names),
            out_names=tuple(out_names),
            lowering_input_output_aliases=(),
            sim_require_finite=True,
            sim_require_nnan=True,
            nc=nc,
        )
        return tuple(outs)

    devices = jax.devices()[:NCORES]
    mesh = Mesh(np.asarray(devices), ("core",))
    n_outs = len(out_names)
    sharded = jax.jit(
        shard_map(
            _body, mesh=mesh,
            in_specs=(PartitionSpec("core"),) * (n_params + n_outs),
            out_specs=(PartitionSpec("core"),) * n_outs,
            check_rep=False,
        ),
        donate_argnums=tuple(range(n_params, n_params + n_outs)),
        keep_unused=True,
    )
    concat_in = [
        np.concatenate([np.asarray(m[nm]) for m in in_maps], axis=0)
        for nm in in_names
    ]
    concat_in = jax.device_put(concat_in)
    zeros = [
        np.zeros((NCORES * z.shape[0], *z.shape[1:]), z.dtype)
        for z in zero_outs
    ]
    # warmup (compile)
    out = sharded(*concat_in, *zeros)
    jax.block_until_ready(out)
    times = []
    for _ in range(iters):
        t0 = time.perf_counter()
        out = sharded(*concat_in, *zeros)
        jax.block_until_ready(out)
        times.append(time.perf_counter() - t0)
    return times
